# revision 26
# baseline (speedup 1.0000x reference)
"""Multi-head attention block (qkv proj -> softmax attention -> out proj)
for B=2, N=2048, C=1024, H=16 heads of d=64, distributed over 8 NeuronCores.

Sharding: core c = (b, g) with b = c // 4 (batch), g = c % 4 (head group of
4 heads). Each core computes q/k/v for its 4 heads, full softmax attention,
and a partial output projection (its 256 input channels of w_proj). The
host sums the 4 per-batch partials and adds b_proj.

Device layout notes (per core):
  - xT [1024, 2048] = x[b].T so the contraction dim (C) lands on SBUF
    partitions for both qkv orientations.
  - q/k are produced transposed ([head_dim, tokens]); consecutive heads sit
    at partition offsets 0 / 64 so the two K=64 score matmuls of a head
    pair occupy disjoint PE row groups and run concurrently (row tiling).
  - v is produced in [tokens, head_dim] layout with an extra all-ones
    column per head; the PV matmul then yields both the unnormalized
    attention output and the softmax denominator Z in one pass.
  - softmax has no max-subtraction: scores are ~N(0,1) (|S*scale| < ~8),
    safely inside fp32 exp range.

Pipeline schedule (the part that matters for time):
  - The ACT engine's exp stream is the pace-setter (~1 us per 2-head x
    512-token step, 128 steps). Everything else must hide under it.
  - Scores/PV/exp use their own psum pools (s: 2x2 banks, pv: 2x1): the
    staging matmuls (qkv / v / proj) rotate through a separate 2x1-bank
    stage pool, so the first score group is NOT chained behind the whole
    qkv phase by pool-slot reuse (that chaining cost the old version a
    52 us serial head).
  - Input DMAs are ordered so the first score group's deps (xT token
    chunk 0, wqk) land first, split across 4 queue engines; ~30 warm-up
    matmuls keep the PE HAM un-throttled during the DMA wait.
  - qkv for the second head pair, v production, and the output proj are
    emitted as low-priority fillers in need-by order; the Tile scheduler
    drops them into PE gaps under the exp pace.
"""

import sys
import types

import numpy as np
import ml_dtypes

B = 2
N = 2048
C = 1024
H = 16
D = 64
HL = H // 4          # heads per core = 4
SCALE = D ** -0.5
N_CORES = 8
KT = C // 128        # 8 contraction tiles
MT = N // 128        # 16 token tiles
BF = ml_dtypes.bfloat16

_cache = {}


def _install_ntff_hook():
    """Register the axon NTFF profiling hook that this image's antenv lacks
    (profiling degrades gracefully without it; needed for exec_time_ns)."""
    try:
        import antenv.axon_hooks  # noqa: F401
        return
    except ImportError:
        pass
    try:
        import antenv
        from trn_agent_boot.trn_boot import _ntff_profile_via_ctypes
    except ImportError:
        return
    mod = types.ModuleType("antenv.axon_hooks")
    _hook = [None]
    mod.set_axon_ntff_profile_hook = lambda h: _hook.__setitem__(0, h)
    mod.get_axon_ntff_profile_hook = lambda: _hook[0]
    sys.modules["antenv.axon_hooks"] = mod
    antenv.axon_hooks = mod
    try:
        mod.set_axon_ntff_profile_hook(
            _ntff_profile_via_ctypes("/opt/axon/libaxon_pjrt.so")
        )
    except Exception:
        pass


def _build_program(v_bias_nonzero: bool, qk_bias_nonzero: bool):
    from contextlib import ExitStack

    import concourse.bass as bass  # noqa: F401
    import concourse.tile as tile
    from concourse import bacc, mybir

    f32 = mybir.dt.float32
    bf16 = mybir.dt.bfloat16
    Exp = mybir.ActivationFunctionType.Exp
    add = mybir.AluOpType.add

    nc = bacc.Bacc("TRN2", target_bir_lowering=False, debug=False,
                   num_devices=N_CORES)

    # inputs are host-prepacked so every DMA moves >=4KB contiguous bytes
    # per partition (small lines collapse per-DMA-engine rate 23 -> 8 GB/s):
    #   xT[mc][p][kt*512+c], wqk[p][kt*512+c], wv[p][kt*256+c]
    xT_d = nc.dram_tensor("xT", [4, 128, KT * 512], bf16,
                          kind="ExternalInput").ap()
    wqk_d = nc.dram_tensor("wqk", [128, KT * 512], bf16,
                           kind="ExternalInput").ap()
    wv_d = nc.dram_tensor("wv", [128, KT * 256], bf16,
                          kind="ExternalInput").ap()
    wp_d = nc.dram_tensor("wp", [256, C], bf16, kind="ExternalInput").ap()
    bqk_d = nc.dram_tensor("bqk", [512, 1], f32, kind="ExternalInput").ap()
    bv_d = nc.dram_tensor("bv", [64, 4], f32, kind="ExternalInput").ap()
    y_d = nc.dram_tensor("y", [N, C], bf16, kind="ExternalOutput").ap()
    warm_d = nc.dram_tensor("warm", [1, 8], f32, kind="ExternalOutput").ap()

    with tile.TileContext(nc) as tc, ExitStack() as ctx:
        persist = ctx.enter_context(tc.tile_pool(name="persist", bufs=1))
        # PSUM budget (8 banks): scores 2x2 + pv 2x1 + staging 2x1.
        # Separate pools so the attention pipeline's slot rotation never
        # chains behind qkv/v/proj staging (and vice versa).
        s_pool = ctx.enter_context(
            tc.tile_pool(name="s", bufs=2, space="PSUM"))
        pv_pool = ctx.enter_context(
            tc.tile_pool(name="pv", bufs=2, space="PSUM"))
        stage_pool = ctx.enter_context(
            tc.tile_pool(name="stage", bufs=2, space="PSUM"))
        es_pool = ctx.enter_context(tc.tile_pool(name="es", bufs=40))
        z_pool = ctx.enter_context(tc.tile_pool(name="z", bufs=3))
        y_pool = ctx.enter_context(tc.tile_pool(name="ysb", bufs=3))
        zd_pool = ctx.enter_context(
            tc.tile_pool(name="zd", bufs=4, space="DRAM"))

        xT = persist.tile([128, 4, KT, 512], bf16)
        wqk = persist.tile([128, KT, 512], bf16)
        wv = persist.tile([128, KT, 256], bf16)
        wp = persist.tile([128, 2, C], bf16)
        bq = persist.tile([128, 4], f32)
        bv = persist.tile([64, 4], f32) if v_bias_nonzero else None
        # q/k activations split into per-(dim-tile, token-chunk) tiles so the
        # scheduler releases attention matmuls as soon as each chunk lands
        qkT = [[persist.tile([128, 512], bf16, name=f"qkT{nt}_{mc}")
                for mc in range(4)] for nt in range(4)]
        v_sb = persist.tile([128, MT, HL * 65], bf16)
        out_sb = persist.tile([128, 2, N], bf16)
        warm_sb = persist.tile([1, 8], f32)
        wrm_mm = persist.tile([128, 128], bf16)

        # ---- input DMA, ordered for the earliest possible first exp ----
        # each transfer is one prepacked block with 4-8KB contiguous rows.
        # The first qk unit streams kt 0-7 in order, so wqk / xT-mc0 are
        # split into kt halves: its first matmuls start after half the
        # bytes. need-by: xT mc-block m at exp-step 4m, wv by steps ~8-30
        # (PV lags exp), wp late (proj).
        xTf = xT[:].rearrange("p a k c -> p (a k c)")
        wqkf = wqk[:].rearrange("p k c -> p (k c)")
        wvf = wv[:].rearrange("p k c -> p (k c)")
        # wave 1 in kt-quarters so the first qk matmuls (which consume kt in
        # order) can start after ~1/4 of the bytes
        for qq in range(4):
            nc.sync.dma_start(xTf[:, qq * 1024:(qq + 1) * 1024],
                              xT_d[0][:, qq * 1024:(qq + 1) * 1024])
            nc.scalar.dma_start(wqkf[:, qq * 1024:(qq + 1) * 1024],
                                wqk_d[:, qq * 1024:(qq + 1) * 1024])
        nc.gpsimd.dma_start(xTf[:, 4096:8192], xT_d[1])
        if qk_bias_nonzero:
            # bqk[512,1] -> [128 partitions, 4 tiles] (first qk drain)
            nc.sync.dma_start(bq[:],
                              bqk_d.rearrange("(t p) o -> p (t o)", p=128))
        nc.sync.dma_start(wvf[:], wv_d[:])
        nc.scalar.dma_start(xTf[:, 8192:12288], xT_d[2])
        nc.sync.dma_start(xTf[:, 12288:16384], xT_d[3])
        for ct in range(2):
            nc.gpsimd.dma_start(wp[:, ct, :], wp_d[ct * 128:(ct + 1) * 128, :])
        if v_bias_nonzero:
            nc.gpsimd.dma_start(bv[:], bv_d[:])

        # ---- warm-ups during the DMA wait ----
        # a few dummy matmuls nudge the PE HAM window toward 2.4 GHz; they
        # must END before the input DMA lands (in-order PE queue), so keep
        # them short. One exp pulls the ACT table load forward.
        nc.vector.memset(wrm_mm[:], 0.0)
        wps = stage_pool.tile([128, 128], f32, tag="st", name="warm")
        for _ in range(24):
            nc.tensor.matmul(wps[:], lhsT=wrm_mm[:, 0:128],
                             rhs=wrm_mm[:, 0:128], start=True, stop=True)
        nc.vector.memset(warm_sb[:], 0.0)
        nc.scalar.activation(warm_sb[:], warm_sb[:], Exp)
        nc.sync.dma_start(warm_d[:], warm_sb[:])

        def qk_block(nt, mcs=range(4)):
            for mc in mcs:
                ps = stage_pool.tile([128, 512], f32, tag="st",
                                     name=f"qk{nt}_{mc}")
                for kt in range(KT):
                    nc.tensor.matmul(
                        ps[:],
                        lhsT=wqk[:, kt, nt * 128:(nt + 1) * 128],
                        rhs=xT[:, mc, kt, :],
                        start=(kt == 0), stop=(kt == KT - 1))
                if qk_bias_nonzero:
                    nc.vector.tensor_scalar(
                        out=qkT[nt][mc][:], in0=ps[:],
                        scalar1=bq[:, nt:nt + 1], scalar2=None, op0=add)
                else:
                    nc.vector.tensor_copy(qkT[nt][mc][:], ps[:])

        def v_block(mts):
            for mt in mts:
                ps = stage_pool.tile([128, 256], f32, tag="st", name=f"v{mt}")
                for kt in range(KT):
                    nc.tensor.matmul(
                        ps[:],
                        lhsT=xT[:, mt // 4, kt,
                                (mt % 4) * 128:(mt % 4 + 1) * 128],
                        rhs=wv[:, kt, :],
                        start=(kt == 0), stop=(kt == KT - 1))
                # v_aug per head = [v | ones]: the PV matmul then puts v at
                # psum partitions 0..63 and the denominator Z at partition 64
                dst = v_sb[:, mt, :].rearrange("p (h c) -> p h c", c=65)
                nc.vector.tensor_copy(
                    dst[:, :, 0:64], ps[:].rearrange("p (h c) -> p h c",
                                                     c=64))
                nc.vector.memset(dst[:, :, 64:65], 1.0)

        # One step = one j-tile, BOTH heads in one 2-bank psum tile
        # (h0 in cols 0:512, h1 in 512:1024). A single exp covers the pair,
        # so the pair's two K=64 row-group matmuls become ready together and
        # schedule adjacently -> they run concurrently in disjoint PE halves.
        NG = MT

        def s_group(step):
            rnd, jt = step // NG, step % NG
            hp, ic = rnd // 4, rnd % 4
            ss = s_pool.tile([128, 1024], f32, tag="s",
                             name=f"s{hp}_{ic}_{jt}")
            for hh in range(2):
                po = hh * 64
                nc.tensor.matmul(
                    ss[:, hh * 512:(hh + 1) * 512],
                    lhsT=qkT[2 + hp][jt // 4][
                        po:po + 64, (jt % 4) * 128:(jt % 4 + 1) * 128],
                    rhs=qkT[hp][ic][po:po + 64, :],
                    start=True, stop=True)
            return ss

        def pv_normalize(hp, ic, pvs):
            for hh in range(2):
                # release the pv psum slot quickly with a single copy, then
                # run the whole normalize chain from SBUF off-critical-path.
                # DVE ops stay lane-aligned; cross-partition moves use DMA.
                pv = pvs[hh]
                oa = z_pool.tile([128, 512], f32, tag="oa")
                nc.vector.tensor_copy(oa[:], pv[:])
                zd = zd_pool.tile([1, 512], f32, tag="zd")
                nc.sync.dma_start(zd[:], oa[64:65, :])
                zbz = z_pool.tile([64, 512], f32, tag="zbz")
                nc.sync.dma_start(
                    zbz[:], zd[0:1, :].to_broadcast([64, 512]))
                zb = z_pool.tile([64, 512], f32, tag="zb")
                nc.vector.reciprocal_approx_fast(zb[:], zbz[:])
                if hh == 0:
                    dst = out_sb[0:64, hp, ic * 512:(ic + 1) * 512]
                else:
                    dst = z_pool.tile([64, 512], bf16, tag="o1")
                nc.vector.tensor_mul(dst, oa[0:64, :], zb[:])
                if v_bias_nonzero:
                    h = 2 * hp + hh
                    nc.vector.tensor_scalar(
                        out=dst, in0=dst, scalar1=bv[0:64, h:h + 1],
                        scalar2=None, op0=add)
                if hh == 1:
                    nc.sync.dma_start(
                        out_sb[64:128, hp, ic * 512:(ic + 1) * 512],
                        dst[:])

        def proj_block(its, pool=None, tag="st", use_act=False):
            for it in its:
                for oc in range(2):
                    ps = (pool or stage_pool).tile([128, 512], f32, tag=tag,
                                                   name=f"y{it}_{oc}")
                    for ct in range(2):
                        nc.tensor.matmul(
                            ps[:],
                            lhsT=out_sb[:, ct, it * 128:(it + 1) * 128],
                            rhs=wp[:, ct, oc * 512:(oc + 1) * 512],
                            start=(ct == 0), stop=(ct == 1))
                    ysb = y_pool.tile([128, 512], bf16, tag="y")
                    # in the tail (after the last exp) ACT is idle: alternate
                    # the psum-drain copy across engines to release slots 2x
                    # faster; during the main phase keep ACT exp-only
                    if use_act and oc == 1:
                        nc.scalar.copy(ysb[:], ps[:])
                    else:
                        nc.vector.tensor_copy(ysb[:], ps[:])
                    nc.sync.dma_start(
                        y_d[it * 128:(it + 1) * 128,
                            oc * 512:(oc + 1) * 512],
                        ysb[:])

        # Program order must be semantic order (Tile deps are program-order
        # RAW/WAR). The first score group only needs (k01, q01) for token
        # chunk 0 -> emit those two staging units at normal priority, and
        # everything else as low-priority fillers in need-by order; the
        # scheduler slots them into PE gaps under the ACT exp pace.
        qk_block(2, [0])               # k for heads 0,1, token chunk 0
        qk_block(0, [0])               # q for heads 0,1, i-chunk 0
        with tc.high_priority(offset=-20000):
            # strict need-by order (in exp steps): k chunks qk2[mc] at step
            # 4mc feed the score stream directly; v[mt] only feeds PV,
            # which lags (round-0 PV sits in a lower band and drains into
            # later rounds' PE gaps, held by the deep es pool); qk0[ic] at
            # 16ic-2; second head pair k/q interleaved at steps 62-110.
            qk_block(2, [1])
            qk_block(2, [2])
            qk_block(2, [3])
            qk_block(0, [1])
            v_block(range(MT))
            qk_block(0, [2])
            qk_block(0, [3])
            qk_block(3, [0])
            qk_block(1, [0])
            qk_block(3, [1])
            qk_block(3, [2])
            qk_block(3, [3])
            qk_block(1, [1])
            qk_block(1, [2])
            qk_block(1, [3])

        # One flat software pipeline across all 8 (hp, ic) rounds. The PV
        # matmuls for step st are emitted in iteration st+1 (one full exp
        # behind): when they reach the head of the in-order PE queue their
        # exp has long finished, so they never head-of-line-block the
        # score stream.
        NSTEP = 8 * NG
        LOOK = 2
        with tc.high_priority():
            ss_q = {i: s_group(i) for i in range(LOOK)}
            es_q = {}
            pvs_by = {}

            def emit_pv(st):
                rnd, jt = st // NG, st % NG
                hp, ic = rnd // 4, rnd % 4
                if jt == 0:
                    pvs_by[rnd] = [pv_pool.tile([128, 512], f32, tag="pv",
                                                name=f"pv{hp}_{ic}_{i}")
                                   for i in range(2)]
                pvs = pvs_by[rnd]
                es = es_q.pop(st)
                for hh in range(2):
                    h = 2 * hp + hh
                    nc.tensor.matmul(
                        pvs[hh][0:65, :],
                        lhsT=v_sb[:, jt, h * 65:(h + 1) * 65],
                        rhs=es[:, hh * 512:(hh + 1) * 512],
                        start=(jt == 0), stop=(jt == MT - 1))
                if jt == NG - 1:
                    pv_normalize(hp, ic, pvs)
                    del pvs_by[rnd]

            for st in range(NSTEP):
                es = es_pool.tile([128, 1024], bf16, tag="es")
                nc.scalar.activation(es[:], ss_q[st % LOOK][:], Exp,
                                     scale=SCALE)
                es_q[st] = es
                if st >= 1:
                    if st - 1 < NG:
                        with tc.high_priority(offset=-5000):
                            emit_pv(st - 1)
                    else:
                        emit_pv(st - 1)
                if st + LOOK < NSTEP:
                    ss_q[st % LOOK] = s_group(st + LOOK)
            emit_pv(NSTEP - 1)

        # proj for i-chunk ic becomes ready after round (hp=1, ic) = step
        # 64+16ic; the low band slots it into late-phase PE gaps without
        # jamming the attention pipeline's in-order PE queue.
        with tc.high_priority(offset=-40000):
            for ic in range(3):
                proj_block(range(ic * 4, (ic + 1) * 4))
        # tail: after the last normalize both psum pools are free and ACT is
        # idle -- fan the last projection chunk over both pools and two
        # copy engines so the drain pipeline never serializes
        proj_block(range(12, 14), use_act=True)
        proj_block(range(14, 16), pool=pv_pool, tag="pv", use_act=True)

    nc.compile()
    return nc


def _prep_inputs(x, w_qkv, b_qkv, w_proj):
    """Build the 8 per-core input maps (host-side shard + transpose + cast)."""
    w3 = w_qkv.reshape(C, 3, H, D)
    b3 = b_qkv.reshape(3, H, D)
    in_maps = []
    for c in range(N_CORES):
        b, g = divmod(c, 4)
        hs = slice(g * HL, (g + 1) * HL)
        wq = w3[:, 0, hs, :].reshape(C, 256)
        wk = w3[:, 1, hs, :].reshape(C, 256)
        wvv = w3[:, 2, hs, :].reshape(C, 256)
        bqh = b3[0, hs, :].reshape(256)
        bkh = b3[1, hs, :].reshape(256)
        bvh = b3[2, hs, :].reshape(256)
        # q/k transposed layout: head pair (2j, 2j+1) shares an SBUF tile
        # with partition offsets 0/64 -> natural [256,1] order is fine:
        # tile t covers dims [t*128,(t+1)*128) = heads 2t,2t+1.
        # xT/wqk/wv are packed (kt p) rows -> [p][kt...] so each DMA block
        # has 4-8KB contiguous per partition (full DMA-engine line rate).
        xt = x[b].T.astype(BF)                       # [C, N]
        xtp = np.ascontiguousarray(
            xt.reshape(KT, 128, 4, 512).transpose(2, 1, 0, 3)
        ).reshape(4, 128, KT * 512)
        wqk2 = np.concatenate([wq, wk], axis=1).astype(BF)
        wqkp = np.ascontiguousarray(
            wqk2.reshape(KT, 128, 512).transpose(1, 0, 2)
        ).reshape(128, KT * 512)
        wvp = np.ascontiguousarray(
            wvv.astype(BF).reshape(KT, 128, 256).transpose(1, 0, 2)
        ).reshape(128, KT * 256)
        in_maps.append({
            "xT": xtp,
            "wqk": wqkp,
            "wv": wvp,
            "wp": w_proj[g * 256:(g + 1) * 256, :].astype(BF),
            "bqk": np.concatenate([bqh, bkh]).reshape(512, 1)
                     .astype(np.float32),
            "bv": np.ascontiguousarray(bvh.reshape(4, 64).T)
                    .astype(np.float32),
        })
    return in_maps


def _get_program(v_bias_nonzero: bool, qk_bias_nonzero: bool):
    key = ("prog", v_bias_nonzero, qk_bias_nonzero)
    if key not in _cache:
        _install_ntff_hook()
        _cache[key] = _build_program(v_bias_nonzero, qk_bias_nonzero)
    return _cache[key]


def run(x, w_qkv, b_qkv, w_proj, b_proj, trace=False, trace_kwargs=None):
    from concourse import bass_utils
    bass_utils.upload_artifacts = lambda tmpdir: tmpdir  # no cloud upload

    x = np.asarray(x, dtype=np.float32)
    w_qkv = np.asarray(w_qkv, dtype=np.float32)
    b_qkv = np.asarray(b_qkv, dtype=np.float32)
    w_proj = np.asarray(w_proj, dtype=np.float32)
    b_proj = np.asarray(b_proj, dtype=np.float32)

    b3 = b_qkv.reshape(3, H, D)
    v_bias_nonzero = bool(np.any(b3[2] != 0.0))
    qk_bias_nonzero = bool(np.any(b3[0] != 0.0) or np.any(b3[1] != 0.0))
    nc = _get_program(v_bias_nonzero, qk_bias_nonzero)
    in_maps = _prep_inputs(x, w_qkv, b_qkv, w_proj)
    res = bass_utils.run_bass_kernel_spmd(
        nc, in_maps, list(range(N_CORES)), trace=trace,
        **(trace_kwargs or {}))

    out = np.zeros((B, N, C), dtype=np.float32)
    for b in range(B):
        acc = np.zeros((N, C), dtype=np.float32)
        for g in range(4):
            acc += res.results[b * 4 + g]["y"].astype(np.float32)
        out[b] = acc + b_proj
    return out, res


def kernel(x, w_qkv, b_qkv, w_proj, b_proj):
    out, _ = run(x, w_qkv, b_qkv, w_proj, b_proj, trace=False)
    return out
